# revision 27
# baseline (speedup 1.0000x reference)
"""Distributed Bass kernel for llama-style GQA attention on 8 trn2 NeuronCores.

Sharding: 2-way data-parallel over batch x 4-way tensor-parallel over heads.
Core c handles batch b=c//4 and head group t=c%4 (8 q-heads, 2 kv-heads).
wq/wk/wv split column-wise per head group; wo split row-wise; each core
produces a partial [S, HIDDEN] bf16 output, host sums the 4 partials per batch.

On-chip flow per core (all matmuls bf16, psum f32):
  xT (pre-transposed on host) @ wqkv -> k|q|v per 128-seq block (512+256 wide
  matmul pair per contraction chunk); RoPE on k and q together in planar
  layout (host pre-permutes wq/wk columns to [real|imag] halves per head) in
  bf16; 1/sqrt(D) is folded into the exp activation scale.
  PE-transpose q,k to [d, seq] (interleaved into later proj blocks to hide
  the single-psum-slot round trip); v kept [seq, d] with appended ones cols.
  scores^T[k,q] = kT.T @ qT ; exp(0.125*s) ; causal handled via aligned
  128x128 tri mask and column-narrowed score and ctx matmuls; the ones
  column gives softmax denominators for free; normalize via gpsimd partition
  broadcast + fast reciprocal; out = sum_c ctxT_c.T @ wo_c accumulated in one
  psum group, stored bf16.
  Schedule: phase A = projections + attention qb groups 0,1 interleaved;
  phase B (proj psum banks released) = qb groups 2,3 as TWO independent
  attention streams (own psum rings, exp batched 1024-wide below the
  diagonal) interleaved with all output-projection units.
"""

import numpy as np
import ml_dtypes

import concourse.bass as bass
import concourse.mybir as mybir
import concourse.tile as tile
from concourse import bacc
from concourse.bass_utils import run_bass_kernel_spmd
from concourse.masks import make_identity

B, S, HID = 2, 2048, 2048
D = 64
NQ, NKV = 8, 2          # per-core heads
KW, QW, VW = NKV * D, NQ * D, NKV * D
QKVW = KW + QW + VW     # 768, layout [k(128) | q(512) | v(128)]
P = 128
SB = S // P             # 16 seq blocks
KC = HID // P           # 16 contraction chunks
F32 = mybir.dt.float32
BF16 = mybir.dt.bfloat16
BF = ml_dtypes.bfloat16
AF = mybir.ActivationFunctionType

_CACHE = {}


def _emit_graph(nc, tc, xT, wqkv, wo, cosb, sinb, out):
    with tc.tile_pool(name="const", bufs=1) as const, \
         tc.tile_pool(name="big", bufs=1) as big:
        # persistent tensors
        qT_sb = [big.tile([P, S], BF16, tag=f"qT{t}", name=f"qT{t}") for t in range(4)]
        kT_sb = [big.tile([P, S], BF16, tag=f"kT{k}", name=f"kT{k}") for k in range(NKV)]
        # vaug layout: per (sb, kv): 65 cols (64 v dims + 1 ones col)
        vaug_sb = big.tile([P, SB * NKV * 65], BF16, tag="va")
        ctxT_sb = [big.tile([P, S], BF16, tag=f"cT{t}", name=f"cT{t}") for t in range(4)]
        rot2 = big.tile([P, SB * 640], BF16, tag="rot2")   # roped k|q per sb
        wqkv_sb = big.tile([P, KC * QKVW], BF16, tag="wqkv")
        wo_sb = big.tile([P, 4 * HID], BF16, tag="wo")
        cos_sb = big.tile([P, SB * 32], BF16, tag="c1")
        sin_sb = big.tile([P, SB * 32], BF16, tag="s1")

        ident = const.tile([P, P], BF16, tag="id")
        make_identity(nc, ident[:, :])
        # tri01[k, q] = 1 where q >= k else 0 (keep-mask for aligned diag blocks)
        tri01 = const.tile([P, P], BF16, tag="tri")
        nc.gpsimd.memset(tri01[:, :], 1.0)
        nc.gpsimd.affine_select(
            out=tri01[:, :], in_=tri01[:, :], compare_op=mybir.AluOpType.is_ge,
            fill=0.0, base=0, pattern=[[1, P]], channel_multiplier=-1,
        )
        nc.gpsimd.memset(vaug_sb[:, :], 1.0)

        with tc.tile_pool(name="xp", bufs=2) as xp_p, \
             tc.tile_pool(name="rt", bufs=2) as rt_p, \
             tc.tile_pool(name="exs", bufs=6) as exs_p, \
             tc.tile_pool(name="nrm", bufs=6) as nrm_p, \
             tc.tile_pool(name="osb", bufs=6) as osb_p:

            # all dram inputs are host-pre-permuted so every load is a
            # contiguous fat-packet 2D copy; the sync DMA queue drains in
            # order, so first xb block, then wqkv in chunks (first proj
            # matmul only needs chunk 0 via subtile deps)
            xb_tiles = {}

            def xb_load(sb, chunks=1):
                xb = xp_p.tile([P, KC * P], BF16, tag="xb", name="xb")
                w = KC * P // chunks
                for c in range(chunks):
                    nc.sync.dma_start(out=xb[:, c * w:(c + 1) * w],
                                      in_=xT[sb * P:(sb + 1) * P, c * w:(c + 1) * w])
                xb_tiles[sb] = xb

            xb_load(0, chunks=4)
            for c in range(4):
                nc.sync.dma_start(out=wqkv_sb[:, c * 4 * QKVW:(c + 1) * 4 * QKVW],
                                  in_=wqkv[:, c * 4 * QKVW:(c + 1) * 4 * QKVW])
            nc.sync.dma_start(out=cos_sb[:, :], in_=cosb)
            nc.sync.dma_start(out=sin_sb[:, :], in_=sinb)

            def rope_and_v(sb, pga, pgb):
                rsrc = rt_p.tile([P, KW + QW], BF16, tag="rsrc", name="rsrc")
                nc.vector.tensor_copy(rsrc[:, 0:512], pga[:, :])
                nc.vector.tensor_copy(rsrc[:, 512:KW + QW], pgb[:, 0:128])
                nh = NKV + NQ
                rs = rsrc[:, :].rearrange("p (h i) -> p h i", h=nh)
                ev, od = rs[:, :, 0:32], rs[:, :, 32:64]
                c = cos_sb[:, sb * 32:(sb + 1) * 32].rearrange("p (o i) -> p o i", o=1).broadcast_to((P, nh, 32))
                s = sin_sb[:, sb * 32:(sb + 1) * 32].rearrange("p (o i) -> p o i", o=1).broadcast_to((P, nh, 32))
                t1 = rt_p.tile([P, nh * 32], BF16, tag="t1", name="t1")
                t2 = rt_p.tile([P, nh * 32], BF16, tag="t2", name="t2")
                t1r = t1[:, :].rearrange("p (h i) -> p h i", h=nh)
                t2r = t2[:, :].rearrange("p (h i) -> p h i", h=nh)
                dst = rot2[:, sb * 640:(sb + 1) * 640].rearrange("p (h i) -> p h i", h=nh)
                dst_e, dst_o = dst[:, :, 0:32], dst[:, :, 32:64]
                nc.vector.tensor_mul(t1r, ev, c)
                nc.vector.tensor_mul(t2r, od, s)
                nc.vector.tensor_sub(dst_e, t1r, t2r)
                nc.vector.tensor_mul(t1r, ev, s)
                nc.vector.tensor_mul(t2r, od, c)
                nc.vector.tensor_add(dst_o, t1r, t2r)
                vdst = vaug_sb[:, sb * 130: sb * 130 + 130].rearrange("p (kv c) -> p kv c", kv=2)[:, :, 0:D]
                vsrc = pgb[:, 128:256].rearrange("p (kv c) -> p kv c", kv=2)
                nc.vector.tensor_copy(vdst, vsrc)

            def wo_unit(pso_p, sb, n, copy_eng="v", tag="po"):
                po = pso_p.tile([P, 512], F32, tag=tag, name="po")
                for c in range(4):
                    nc.tensor.matmul(po[:, :], ctxT_sb[c][:, sb * P:(sb + 1) * P],
                                     wo_sb[:, c * HID + n * 512: c * HID + (n + 1) * 512],
                                     start=(c == 0), stop=(c == 3))
                ob = osb_p.tile([P, 512], BF16, tag="ob", name="ob")
                if copy_eng == "s":
                    nc.scalar.activation(ob[:, :], po[:, :], AF.Copy)
                else:
                    nc.vector.tensor_copy(ob[:, :], po[:, :])
                nc.sync.dma_start(out=out[sb * P:(sb + 1) * P, n * 512:(n + 1) * 512],
                                  in_=ob[:, :])

            def normalize(ctx, t, roff, qb):
                den = nrm_p.tile([1, 512], F32, tag="den", name="den")
                nc.vector.tensor_copy(den[:, :], ctx[64:65, :])
                dbc = nrm_p.tile([D, 512], F32, tag="dbc", name="dbc")
                nc.gpsimd.partition_broadcast(dbc[:, :], den[:, :])
                rbc = nrm_p.tile([D, 512], F32, tag="rbc", name="rbc")
                nc.vector.reciprocal_approx_fast(out=rbc[:, :], in_=dbc[:, :])
                ntmp = nrm_p.tile([D, 512], BF16, tag="ntmp", name="ntmp")
                nc.vector.tensor_mul(ntmp[:, :], ctx[0:D, :], rbc[:, :])
                nc.sync.dma_start(out=ctxT_sb[t][roff:roff + D, qb * 512:(qb + 1) * 512],
                                  in_=ntmp[:, :])

            # ======== phase A: projections + attention groups 0,1 ========
            with tc.tile_pool(name="psg", bufs=2, space="PSUM") as psg_p, \
                 tc.tile_pool(name="pst", bufs=1, space="PSUM") as pst_p, \
                 tc.tile_pool(name="pssA", bufs=2, space="PSUM") as pssA_p, \
                 tc.tile_pool(name="pscA", bufs=1, space="PSUM") as pscA_p:

                def transpose_closures(sb):
                    def t_k():
                        pt = pst_p.tile([P, P], BF16, tag="pt", name="pt")
                        nc.tensor.transpose(pt[:, :], rot2[:, sb * 640: sb * 640 + P], ident[:, :])
                        nc.vector.tensor_copy(kT_sb[0][0:D, sb * P:(sb + 1) * P], pt[0:D, :])
                        nc.vector.tensor_copy(kT_sb[1][D:P, sb * P:(sb + 1) * P], pt[D:P, :])
                        nc.sync.dma_start(out=kT_sb[0][D:P, sb * P:(sb + 1) * P],
                                          in_=kT_sb[0][0:D, sb * P:(sb + 1) * P])
                        nc.sync.dma_start(out=kT_sb[1][0:D, sb * P:(sb + 1) * P],
                                          in_=kT_sb[1][D:P, sb * P:(sb + 1) * P])

                    def t_q(t):
                        def f():
                            pt = pst_p.tile([P, P], BF16, tag="pt", name="pt")
                            nc.tensor.transpose(pt[:, :], rot2[:, sb * 640 + KW + t * P: sb * 640 + KW + (t + 1) * P],
                                                ident[:, :])
                            nc.vector.tensor_copy(qT_sb[t][:, sb * P:(sb + 1) * P], pt[:, :])
                        return f
                    return [t_k] + [t_q(t) for t in range(4)]

                def proj(sb, tq=()):
                    if sb not in xb_tiles:
                        xb_load(sb)
                    xb = xb_tiles.pop(sb)
                    if sb + 1 <= 15 and sb + 1 not in xb_tiles:
                        xb_load(sb + 1)   # prefetch next block
                    pga = psg_p.tile([P, 512], F32, tag="pga", name="pga")
                    pgb = psg_p.tile([P, 256], F32, tag="pgb", name="pgb")
                    ti = 0
                    for kc in range(KC):
                        nc.tensor.matmul(pga[:, :], xb[:, kc * P:(kc + 1) * P],
                                         wqkv_sb[:, kc * QKVW:kc * QKVW + 512],
                                         start=(kc == 0), stop=(kc == KC - 1))
                        nc.tensor.matmul(pgb[:, :], xb[:, kc * P:(kc + 1) * P],
                                         wqkv_sb[:, kc * QKVW + 512:(kc + 1) * QKVW],
                                         start=(kc == 0), stop=(kc == KC - 1))
                        if ti < len(tq) and kc in (3, 6, 9, 12, 15):
                            tq[ti]()
                            ti += 1
                    while ti < len(tq):
                        tq[ti]()
                        ti += 1
                    # stream in a quarter of wo during mid phase A (needed at
                    # phase B start; too big to load in one go at that point)
                    if sb in (2, 4, 6, 8):
                        c = (sb - 2) // 2
                        nc.sync.dma_start(out=wo_sb[:, c * HID:(c + 1) * HID],
                                          in_=wo[:, c * HID:(c + 1) * HID])
                    rope_and_v(sb, pga, pgb)

                def attn_unit_a(h, qb):
                    t, roff, kv = h // 2, D * (h % 2), h // 4
                    qT = qT_sb[t][roff:roff + D, :]
                    kT = kT_sb[kv][roff:roff + D, :]
                    ctx = pscA_p.tile([65, 512], F32, tag="ctx", name="ctx")
                    nkb = 4 * qb + 4
                    for kb in range(nkb):
                        j0 = max(kb - 4 * qb, 0)
                        sT = pssA_p.tile([P, 512], F32, tag="sT", name="sT")
                        nc.tensor.matmul(sT[:, j0 * P:512], kT[:, kb * P:(kb + 1) * P],
                                         qT[:, qb * 512 + j0 * P:(qb + 1) * 512], start=True, stop=True)
                        ex = exs_p.tile([P, 512], BF16, tag="ex", name="ex")
                        nc.scalar.activation(ex[:, j0 * P:512], sT[:, j0 * P:512], AF.Exp, scale=0.125)
                        if kb >= 4 * qb:
                            nc.vector.tensor_mul(ex[:, j0 * P:(j0 + 1) * P],
                                                 ex[:, j0 * P:(j0 + 1) * P], tri01[:, :])
                        nc.tensor.matmul(ctx[:, j0 * P:512],
                                         vaug_sb[:, kb * 130 + kv * 65: kb * 130 + (kv + 1) * 65],
                                         ex[:, j0 * P:512], start=(kb == 0), stop=(kb == nkb - 1),
                                         skip_group_check=True)
                    normalize(ctx, t, roff, qb)

                # proj 0..5 (transposes of sb-2 ride inside proj sb)
                proj(0)
                proj(1)
                for sb in (2, 3, 4, 5):
                    proj(sb, transpose_closures(sb - 2))
                # group 0 (qb=0) with proj 6..13
                a = [(h, 0) for h in range(8)]
                for i, sb in enumerate(range(6, 14)):
                    attn_unit_a(*a[i])
                    proj(sb, transpose_closures(sb - 2))
                # group 1 (qb=1) with proj 14,15 and transposes 12..15
                a = [(h, 1) for h in range(8)]
                attn_unit_a(*a[0])
                proj(14, transpose_closures(12))
                attn_unit_a(*a[1])
                proj(15, transpose_closures(13))
                attn_unit_a(*a[2])
                t14, t15 = transpose_closures(14), transpose_closures(15)
                for f in t14[:3]:
                    f()
                attn_unit_a(*a[3])
                for f in t14[3:] + t15[:1]:
                    f()
                attn_unit_a(*a[4])
                for f in t15[1:4]:
                    f()
                attn_unit_a(*a[5])
                for f in t15[4:]:
                    f()
                attn_unit_a(*a[6])
                attn_unit_a(*a[7])

            # ======== phase B: attention groups 2,3 dual-stream + wo ========
            with tc.tile_pool(name="ps2a", bufs=1, space="PSUM") as ps2a_p, \
                 tc.tile_pool(name="ps2b", bufs=1, space="PSUM") as ps2b_p, \
                 tc.tile_pool(name="psca", bufs=1, space="PSUM") as psca_p, \
                 tc.tile_pool(name="pscb", bufs=1, space="PSUM") as pscb_p, \
                 tc.tile_pool(name="pso", bufs=2, space="PSUM") as pso_p:

                class Stream:
                    def __init__(self, pss, psc, units):
                        self.pss, self.psc, self.units = pss, psc, list(units)
                        self.ui, self.kb, self.active = 0, 0, False
                        self.done_qb2 = False

                    def step_scores(self):
                        """Emit the next scores pair + exp; return a closure
                        emitting the matching ctx pair (and normalize at unit
                        end), or None when the stream is exhausted."""
                        if not self.active:
                            if self.ui >= len(self.units):
                                return None
                            h, qb = self.units[self.ui]
                            self.t, self.roff, self.kv = h // 2, D * (h % 2), h // 4
                            self.qb = qb
                            self.ctx = self.psc.tile([65, 512], F32, tag="ctx", name="ctx")
                            self.nkb = 4 * qb + 4
                            self.kb = 0
                            self.active = True
                        kb, qb, t, roff, kv = self.kb, self.qb, self.t, self.roff, self.kv
                        ctx, nkb = self.ctx, self.nkb
                        sT = self.pss.tile([P, 1024], F32, tag="sT", name="sT")
                        ex = exs_p.tile([P, 1024], BF16, tag="ex", name="ex")
                        qT = qT_sb[t][roff:roff + D, :]
                        kT = kT_sb[kv][roff:roff + D, :]
                        for half, k in ((0, kb), (1, kb + 1)):
                            j0 = max(k - 4 * qb, 0)
                            nc.tensor.matmul(sT[:, half * 512 + j0 * P:(half + 1) * 512],
                                             kT[:, k * P:(k + 1) * P],
                                             qT[:, qb * 512 + j0 * P:(qb + 1) * 512],
                                             start=True, stop=True)
                        # one exp covering both halves; on diagonal rounds it
                        # also covers the stale gap columns, which are never
                        # read back from ex
                        j0a = max(kb - 4 * qb, 0)
                        nc.scalar.activation(ex[:, j0a * P:1024], sT[:, j0a * P:1024],
                                             AF.Exp, scale=0.125)
                        last = kb + 2 >= nkb
                        ui_next = self.ui + 1

                        def ctx_close():
                            for half, k in ((0, kb), (1, kb + 1)):
                                j0 = max(k - 4 * qb, 0)
                                if k >= 4 * qb:
                                    nc.vector.tensor_mul(
                                        ex[:, half * 512 + j0 * P: half * 512 + (j0 + 1) * P],
                                        ex[:, half * 512 + j0 * P: half * 512 + (j0 + 1) * P],
                                        tri01[:, :])
                                nc.tensor.matmul(ctx[:, j0 * P:512],
                                                 vaug_sb[:, k * 130 + kv * 65: k * 130 + (kv + 1) * 65],
                                                 ex[:, half * 512 + j0 * P:(half + 1) * 512],
                                                 start=(k == 0), stop=(k == nkb - 1),
                                                 skip_group_check=True)
                            if last:
                                normalize(ctx, t, roff, qb)
                                if ui_next == 4:
                                    self.done_qb2 = True

                        if last:
                            self.ui += 1
                            self.active = False
                        else:
                            self.kb += 2
                        return ctx_close

                # stream A: even heads, stream B: odd heads; B staggered 3
                # rounds ahead so diagonal (exp-heavy) tails overlap the other
                # stream's wide body rounds; B's ctx lags one extra round
                sA = Stream(ps2a_p, psca_p, [(h, qb) for qb in (2, 3) for h in (0, 2, 4, 6)])
                sB = Stream(ps2b_p, pscb_p, [(h, qb) for qb in (2, 3) for h in (1, 3, 5, 7)])

                wos = [(sb, n) for sb in range(16) for n in range(4)]
                wi = 0

                def wo_avail():
                    # sb 0..7 anytime in phase B; 8..11 once qb=2 fully done
                    if wi < 32:
                        return True
                    if wi < 48:
                        return sA.done_qb2 and sB.done_qb2
                    return False

                cb_prev = None
                for _ in range(4):   # prime B
                    c = sB.step_scores()
                    if cb_prev:
                        cb_prev()
                    cb_prev = c
                while True:
                    ca = sA.step_scores()
                    cb = sB.step_scores()
                    if ca is None and cb is None and cb_prev is None:
                        break
                    if wi < len(wos) and wo_avail():
                        wo_unit(pso_p, *wos[wi])
                        wi += 1
                    if cb_prev:
                        cb_prev()
                    if ca:
                        ca()
                    cb_prev = cb
                if cb_prev:
                    cb_prev()
                # tail wo units: rotate psum across the now-idle attention
                # pools' slots (6-deep ring) and alternate copies between the
                # idle scalar and vector engines
                tail_slots = [(pso_p, "po"), (psca_p, "ctx"), (ps2a_p, "sT"),
                              (pso_p, "po"), (pscb_p, "ctx"), (ps2b_p, "sT")]
                for i, u in enumerate(wos[wi:]):
                    pool, tag = tail_slots[i % len(tail_slots)]
                    wo_unit(pool, *u, copy_eng=("s" if i % 2 else "v"), tag=tag)


def _build():
    nc = bacc.Bacc("TRN2", target_bir_lowering=False, debug=False, num_devices=8)
    # all inputs host-pre-permuted into DMA-friendly layouts (fat packets):
    # xT[sb*P+p, kc*P+j] = x.T[kc*P+p, sb*P+j]; weights/tables partition-major
    xT = nc.dram_tensor("xT", [S, HID], BF16, kind="ExternalInput").ap()
    wqkv = nc.dram_tensor("wqkv", [P, KC * QKVW], BF16, kind="ExternalInput").ap()
    wo = nc.dram_tensor("wo", [P, 4 * HID], BF16, kind="ExternalInput").ap()
    cosb = nc.dram_tensor("cosb", [P, SB * 32], BF16, kind="ExternalInput").ap()
    sinb = nc.dram_tensor("sinb", [P, SB * 32], BF16, kind="ExternalInput").ap()
    out = nc.dram_tensor("out", [S, HID], BF16, kind="ExternalOutput").ap()
    with tile.TileContext(nc) as tc:
        _emit_graph(nc, tc, xT, wqkv, wo, cosb, sinb, out)
    nc.finalize()
    return nc


def _planar(w, nheads):
    """Permute each head's 64 cols from interleaved (r0,i0,r1,i1,...) to
    planar (r0..r31, i0..i31)."""
    h = w.reshape(w.shape[0], nheads, 32, 2)
    return np.ascontiguousarray(h.transpose(0, 1, 3, 2).reshape(w.shape[0], nheads * 64))


def kernel(x, wq, wk, wv, wo, freqs_cos, freqs_sin, mask):
    x = np.asarray(x, dtype=np.float32)
    wq = np.asarray(wq, dtype=np.float32)
    wk = np.asarray(wk, dtype=np.float32)
    wv = np.asarray(wv, dtype=np.float32)
    wo = np.asarray(wo, dtype=np.float32)
    fc = np.asarray(freqs_cos, dtype=np.float32)
    fs = np.asarray(freqs_sin, dtype=np.float32)

    if "nc" not in _CACHE:
        _CACHE["nc"] = _build()
    nc = _CACHE["nc"]

    wqp = _planar(wq, 32)   # planar per q head
    wkp = _planar(wk, 8)    # planar per kv head
    cosb = np.ascontiguousarray(fc.reshape(SB, P, 32).transpose(1, 0, 2).reshape(P, SB * 32)).astype(BF)
    sinb = np.ascontiguousarray(fs.reshape(SB, P, 32).transpose(1, 0, 2).reshape(P, SB * 32)).astype(BF)
    in_maps = []
    for core in range(8):
        b, t = core // 4, core % 4
        # xT[sb*P+p, kc*P+j] = x[b][sb*P+j, kc*P+p]
        xTb = x[b].reshape(SB, P, KC, P).transpose(0, 3, 2, 1).reshape(S, HID)
        wqkv_full = np.concatenate(
            [wkp[:, t * KW:(t + 1) * KW],
             wqp[:, t * QW:(t + 1) * QW],
             wv[:, t * VW:(t + 1) * VW]], axis=1)   # [HID, 768]
        wqkv_p = wqkv_full.reshape(KC, P, QKVW).transpose(1, 0, 2).reshape(P, KC * QKVW)
        wo_p = wo[t * QW:(t + 1) * QW, :].reshape(4, P, HID).transpose(1, 0, 2).reshape(P, 4 * HID)
        in_maps.append({
            "xT": np.ascontiguousarray(xTb).astype(BF),
            "wqkv": np.ascontiguousarray(wqkv_p).astype(BF),
            "wo": np.ascontiguousarray(wo_p).astype(BF),
            "cosb": cosb, "sinb": sinb,
        })
    trace = bool(_CACHE.get("trace"))
    try:
        res = run_bass_kernel_spmd(nc, in_maps, list(range(8)), trace=trace)
    except Exception:
        if not trace:
            raise
        res = run_bass_kernel_spmd(nc, in_maps, list(range(8)))
    _CACHE["last_result"] = res
    outs = [np.asarray(r["out"], dtype=np.float32) for r in res.results]
    full = np.stack([outs[0] + outs[1] + outs[2] + outs[3],
                     outs[4] + outs[5] + outs[6] + outs[7]], axis=0)
    return full
